# revision 34
# baseline (speedup 1.0000x reference)
"""RealFormer-style MultiHeadAttention on 8 Trainium2 NeuronCores.

Reference computation (B=8, S=1024, D=1024, H=16, HD=64):
    q = split_heads(hidden @ Wq + bq); k = ...; v = ...
    scores = (q @ k^T) * HD**-0.5 + attn_mask + prev_attn_weights
    out    = merge_heads(softmax(scores) @ v)

Sharding: pure data-parallel over batch — one batch element per core,
no collectives.

Per-core kernel design (all matmul operands fp16, accumulation fp32):
  * Host folds SCALE into Wq, pre-transposes hidden, and ships
    E = exp(prevT + maskT - 4) in fp16 — turning the additive RealFormer
    residual into a multiplicative factor on the softmax numerator:
    exp(qk + prev - 8) = exp(qk - 4) * E.  This removes the PE inject
    matmul entirely; the multiply runs on the otherwise-idle DVE.
  * The Activation engine (exp over all S*S*H scores, ~133us busy) must
    never stall PE and must start early.  The schedule therefore begins
    scoring head 0 as soon as q/k block 0 is projected (~14us in), and
    all remaining projection work (v, q/k blocks t+1) is spread as PE
    "fill" between score tiles so PE's per-tile cadence stays above the
    exp() drain rate.  ctx for head h-2 is also interleaved per tile.
  * Per head: scoresT[k,q] = kT^T @ qT into PSUM; ScalarE writes
    exp(scoresT - 4) to fp16 SBUF; DVE multiplies by E in place.  The
    constant shift cancels in the normalization (row sums come free via
    a ones column in vx), so no row-max pass is needed.
  * ctxT[65, q] = vx^T @ probsT accumulated over k; DMA-xbar transpose
    to [q, 65]; VectorE reciprocal + per-partition scale; per-head
    output DMA so writeback overlaps compute.
"""

import sys

if "/opt/trn_rl_repo" not in sys.path:
    sys.path.insert(0, "/opt/trn_rl_repo")

import numpy as np

B, S, D, H = 8, 1024, 1024, 16
HD = D // H
SCALE = HD**-0.5
P = 128
N_CORES = 8
EXP_SHIFT = 4.0

_compiled = {}


def _build(use_bias: bool, reps: int = 1):
    import concourse.bacc as bacc
    import concourse.mybir as mybir
    import concourse.tile as tile

    f16 = mybir.dt.float16
    f32 = mybir.dt.float32
    Exp = mybir.ActivationFunctionType.Exp

    nc = bacc.Bacc("TRN2", target_bir_lowering=False, debug=False)

    hT_d = nc.dram_tensor("hiddenT", (D, S), f16, kind="ExternalInput").ap()
    # wq/wk are shipped pre-sliced by output-dim block t so each per-t
    # weight DMA is fully contiguous: [t, ki, ko*128+c]
    wqs_d = {
        name: nc.dram_tensor(name, (8, P, D), f16, kind="ExternalInput").ap()
        for name in ("wqs", "wks")
    }
    wv_d = nc.dram_tensor("wv", (D, D), f16, kind="ExternalInput").ap()
    em_d = nc.dram_tensor("em", (H, S, S), f16, kind="ExternalInput").ap()
    b_d = {}
    if use_bias:
        b_d = {
            name: nc.dram_tensor(name, (1, D), f16, kind="ExternalInput").ap()
            for name in ("bq", "bk", "bv")
        }
    out_d = nc.dram_tensor("out", (S, D), f32, kind="ExternalOutput").ap()
    out_v = out_d.rearrange("(qo qi) d -> qi qo d", qi=P)

    with tile.TileContext(nc) as tc:
        with (
            tc.tile_pool(name="big", bufs=1) as big,
            tc.tile_pool(name="wqk", bufs=4) as wqk_pool,
            tc.tile_pool(name="wv", bufs=8) as wv_pool,
            tc.tile_pool(name="ppool", bufs=6) as ppool,
            tc.tile_pool(name="probs", bufs=3) as probs_pool,
            tc.tile_pool(name="small", bufs=3) as small,
            tc.tile_pool(name="outp", bufs=3) as outp,
            tc.tile_pool(name="const", bufs=1) as const_pool,
            tc.tile_pool(name="ps_sc", bufs=2, space="PSUM") as ps_sc,
            tc.tile_pool(name="ps_ctx", bufs=1, space="PSUM") as ps_ctx,
            tc.tile_pool(name="ps_t", bufs=2, space="PSUM") as ps_t,
        ):
            for _rep in range(reps):
                neg_shift = const_pool.tile([P, 1], f32, name="negs")
                nc.any.memset(neg_shift, -EXP_SHIFT)
                ident = const_pool.tile([P, P], f16, name="ident")
                from concourse.masks import make_identity

                make_identity(nc, ident)
                if use_bias:
                    ones_row = const_pool.tile([1, 512], f16, name="ones")
                    nc.any.memset(ones_row, 1.0)
                    b_sb = {}
                    for name in ("bq", "bk", "bv"):
                        bt = const_pool.tile([1, D], f16, name=f"bsb_{name}")
                        nc.sync.dma_start(bt, b_d[name])
                        b_sb[name] = bt

                hidT = big.tile([P, 8, S], f16, tag="hidT")

                qT = big.tile([P, 8, S], f16, tag="qT")
                kT = big.tile([P, 8, S], f16, tag="kT")
                vx = big.tile([P, 8, H * 65], f16, tag="vx")
                vx_view = vx.rearrange("p t (h c) -> p t h c", c=65)
                nc.any.memset(vx_view[:, :, :, 64], 1.0)

                probsT_live = {}
                ctx_ps_live = {}
                fill_q = []

                def queue_proj(pname, t, dest):
                    # weight slice DMA issues now (prefetch); the 16 matmuls
                    # + PSUM->SBUF copy go into the PE fill queue
                    wsl = wqk_pool.tile(
                        [P, 8, P], f16, tag="wsl", name=f"wsl_{pname}{t}"
                    )
                    nc.sync.dma_start(
                        wsl,
                        wqs_d["w" + pname + "s"][t].rearrange(
                            "p (ko c) -> p ko c", c=P
                        ),
                    )
                    holder = {}
                    for half in range(2):
                        hs = slice(half * 512, half * 512 + 512)
                        for kt in range(8):

                            def mm(half=half, hs=hs, kt=kt):
                                if "pt" not in holder:
                                    holder["pt"] = ps_sc.tile(
                                        [P, S], f32, tag="pssc", name=f"pp_{pname}{t}"
                                    )
                                nc.tensor.matmul(
                                    holder["pt"][:, hs],
                                    lhsT=wsl[:, kt, :],
                                    rhs=hidT[:, kt, hs],
                                    start=(kt == 0),
                                    stop=(kt == 7 and not use_bias),
                                )

                            fill_q.append(mm)
                        if use_bias:

                            def mmb(hs=hs):
                                nc.tensor.matmul(
                                    holder["pt"][:, hs],
                                    lhsT=b_sb["b" + pname][:, t * P : (t + 1) * P],
                                    rhs=ones_row,
                                    start=False,
                                    stop=True,
                                )

                            fill_q.append(mmb)

                    def cp():
                        nc.vector.tensor_copy(dest[:, t, :], holder["pt"])

                    fill_q.append(cp)

                def queue_v():
                    wts = []
                    for kt in range(8):
                        wt = wv_pool.tile([P, D], f16, tag="w", name=f"w_v{kt}")
                        nc.sync.dma_start(wt, wv_d[kt * P : (kt + 1) * P, :])
                        wts.append(wt)
                    for pt_i in range(8):
                        holder = {}
                        for half in range(2):
                            hs = slice(half * 512, half * 512 + 512)
                            for dt in range(8):

                                def mm(pt_i=pt_i, hs=hs, dt=dt, holder=holder):
                                    if "pv" not in holder:
                                        holder["pv"] = ps_sc.tile(
                                            [P, D], f32, tag="pssc", name=f"pv_{pt_i}"
                                        )
                                    nc.tensor.matmul(
                                        holder["pv"][:, hs],
                                        lhsT=hidT[:, dt, pt_i * P : (pt_i + 1) * P],
                                        rhs=wts[dt][:, hs],
                                        start=(dt == 0),
                                        stop=(dt == 7 and not use_bias),
                                    )

                                fill_q.append(mm)
                            if use_bias:

                                def mmb(hs=hs, holder=holder):
                                    nc.tensor.matmul(
                                        holder["pv"][:, hs],
                                        lhsT=ones_row[:, :P],
                                        rhs=b_sb["bv"][:, hs],
                                        start=False,
                                        stop=True,
                                    )

                                fill_q.append(mmb)

                        def cp(pt_i=pt_i, holder=holder):
                            nc.vector.tensor_copy(
                                vx_view[:, pt_i, :, 0:64],
                                holder["pv"].rearrange("p (h e) -> p h e", e=64),
                            )

                        fill_q.append(cp)

                def emit_ctx_mm(hc, idx):
                    half, ktc = idx // 8, idx % 8
                    hs = slice(half * 512, half * 512 + 512)
                    nc.tensor.matmul(
                        ctx_ps_live[hc][:, hs],
                        lhsT=vx[:, ktc, hc * 65 : (hc + 1) * 65],
                        rhs=probsT_live[hc][:, ktc, hs],
                        start=(ktc == 0),
                        stop=(ktc == 7),
                    )

                def emit_ctx_tail(hc):
                    probsT_live.pop(hc)
                    pc = ctx_ps_live.pop(hc)
                    ctxT_sb = small.tile([65, S], f16, tag="ctxT", name=f"ctxT_{hc}")
                    nc.vector.tensor_copy(ctxT_sb, pc)
                    oh = outp.tile([P, 8, 64], f32, tag="outh", name=f"out_{hc}")
                    for qt in range(8):
                        # PE transpose into the ctx PSUM pool (pc is already
                        # freed by the copy above, so rotation is clean)
                        ptt = ps_t.tile([P, 65], f16, tag="pst", name=f"pt_{hc}_{qt}")
                        nc.tensor.matmul(
                            ptt,
                            lhsT=ctxT_sb[:, qt * P : (qt + 1) * P],
                            rhs=ident[0:65, 0:65],
                            is_transpose=True,
                        )
                        rc = small.tile([P, 1], f32, tag="recip", name=f"rc_{hc}_{qt}")
                        nc.vector.reciprocal(rc, ptt[:, 64:65])
                        nc.vector.tensor_scalar_mul(oh[:, qt, :], ptt[:, 0:64], rc)
                    nc.sync.dma_start(out_v[:, :, hc * 64 : (hc + 1) * 64], oh)

                def emit_head(h, hc, hc2=None, tail2=False, self_ctx=False, mult_chunk=4):
                    # scores for head h; ctx matmuls for head hc (=h-2) and
                    # optionally hc2 interleaved; PE fill (projection) work
                    # drained between score tiles.  self_ctx (last head)
                    # interleaves this head's own ctx half-0, lagged 4 slots
                    # behind the per-tile E-multiply.
                    r, t = h % 2, h // 2
                    rs = slice(r * 64, (r + 1) * 64)
                    ev_ap = em_d[h].rearrange("(ko ki) q -> ki ko q", ki=P)
                    em_sb = []
                    for j in range(2):
                        ej = ppool.tile([P, 4, S], f16, tag="prev", name=f"em_{h}_{j}")
                        eng = nc.sync if j == 0 else nc.gpsimd
                        eng.dma_start(ej, ev_ap[:, j * 4 : (j + 1) * 4, :])
                        em_sb.append(ej)

                    probsT = probs_pool.tile(
                        [P, 8, S], f16, tag="probsT", name=f"probsT_{h}"
                    )
                    probsT_live[h] = probsT
                    if hc is not None:
                        ctx_ps_live[hc] = ps_ctx.tile(
                            [65, S], f32, tag="psc", name=f"ps_c_{hc}"
                        )
                    if hc2 is not None:
                        # second ctx stream's PSUM comes from the score pool
                        # (the dedicated ctx slot is held by hc)
                        ctx_ps_live[hc2] = ps_sc.tile(
                            [65, S], f32, tag="pssc", name=f"ps_c_{hc2}"
                        )
                    n_kt = 8
                    for kt in range(n_kt):
                        ks = slice(kt * P, (kt + 1) * P)
                        ps = ps_sc.tile([P, S], f32, tag="pssc", name=f"ps_s_{h}_{kt}")
                        for half in range(2):
                            hs = slice(half * 512, half * 512 + 512)
                            nc.tensor.matmul(
                                ps[:, hs],
                                lhsT=kT[rs, t, ks],
                                rhs=qT[rs, t, hs],
                                start=True,
                                stop=True,
                            )
                        nc.scalar.activation(
                            probsT[:, kt, :], ps[:], Exp, bias=neg_shift
                        )
                        if kt % mult_chunk == mult_chunk - 1:
                            j0 = kt + 1 - mult_chunk
                            cv = probsT[:, j0 : kt + 1, :]
                            ej = em_sb[j0 // 4]
                            nc.vector.tensor_mul(
                                cv, cv, ej[:, j0 % 4 : j0 % 4 + mult_chunk, :]
                            )
                        if hc is not None:
                            emit_ctx_mm(hc, 2 * kt)
                            emit_ctx_mm(hc, 2 * kt + 1)
                        if hc2 is not None and not self_ctx:
                            emit_ctx_mm(hc2, 2 * kt)
                            emit_ctx_mm(hc2, 2 * kt + 1)
                        if self_ctx and kt >= 4:
                            emit_ctx_mm(h, 2 * (kt - 4))
                            emit_ctx_mm(h, 2 * (kt - 4) + 1)
                        # drain PE fill work (projections) for this slot
                        rem = len(fill_q)
                        if rem:
                            rate = -(-rem // (n_kt - kt))
                            rate = min(rate, 11)
                            for _ in range(min(rate, rem)):
                                fill_q.pop(0)()
                    if hc is not None:
                        emit_ctx_tail(hc)
                    if tail2 and hc2 is not None:
                        emit_ctx_tail(hc2)

                # --- schedule ---
                # q0/k0 projected bulk (PE's first work after hidT lands);
                # their weight DMAs are issued before the big hidT transfer
                queue_proj("q", 0, qT)
                queue_proj("k", 0, kT)
                hT_v = hT_d.rearrange("(do di) s -> di do s", di=P)
                nc.sync.dma_start(hidT[:, :, 0:512], hT_v[:, :, 0:512])
                nc.sync.dma_start(hidT[:, :, 512:S], hT_v[:, :, 512:S])
                while fill_q:
                    fill_q.pop(0)()
                queue_v()
                queue_proj("q", 1, qT)
                queue_proj("k", 1, kT)
                for h in range(16):
                    t = h // 2
                    if h >= 2 and h % 2 == 0 and t + 1 <= 7:
                        queue_proj("q", t + 1, qT)
                    if h >= 2 and h % 2 == 1 and t + 1 <= 7:
                        queue_proj("k", t + 1, kT)
                    emit_head(h, h - 2 if h >= 2 else None,
                              mult_chunk=2 if h == 15 else 4)
                ctx_ps_live[14] = ps_ctx.tile([65, S], f32, tag="psc", name="ps_c_14")
                for idx in range(16):
                    emit_ctx_mm(14, idx)
                emit_ctx_tail(14)
                ctx_ps_live[15] = ps_ctx.tile([65, S], f32, tag="psc", name="ps_c_15")
                for idx in range(8):
                    emit_ctx_mm(15, idx)
                # ctx15: copy finished half-0 while half-1 matmuls run
                pc15 = ctx_ps_live[15]
                ctxT_sb = small.tile([65, S], f16, tag="ctxT", name="ctxT_15")
                nc.vector.tensor_copy(ctxT_sb[:, 0:512], pc15[:, 0:512])
                for idx in range(8, 16):
                    emit_ctx_mm(15, idx)
                nc.vector.tensor_copy(ctxT_sb[:, 512:S], pc15[:, 512:S])
                oh = outp.tile([P, 8, 64], f32, tag="outh", name="out_15")
                for qt in range(8):
                    ptt = ps_t.tile([P, 65], f16, tag="pst", name=f"pt_15_{qt}")
                    nc.tensor.matmul(
                        ptt,
                        lhsT=ctxT_sb[:, qt * P : (qt + 1) * P],
                        rhs=ident[0:65, 0:65],
                        is_transpose=True,
                    )
                    rc = small.tile([P, 1], f32, tag="recip", name=f"rc_15_{qt}")
                    nc.vector.reciprocal(rc, ptt[:, 64:65])
                    nc.vector.tensor_scalar_mul(oh[:, qt, :], ptt[:, 0:64], rc)
                nc.sync.dma_start(out_v[:, :, 15 * 64 : 16 * 64], oh)
                probsT_live.pop(15)
                ctx_ps_live.pop(15)

    nc.compile()
    return nc


def _get_compiled(use_bias: bool, reps: int = 1):
    key = (use_bias, reps)
    if key not in _compiled:
        _compiled[key] = _build(use_bias, reps)
    return _compiled[key]


def _prepare_in_maps(
    hidden_states, attn_mask, prev_attn_weights, Wq, bq, Wk, bk, Wv, bv, use_bias
):
    hs = np.asarray(hidden_states, np.float32)
    mask = np.asarray(attn_mask, np.float32)
    prev = np.asarray(prev_attn_weights, np.float32)

    wq16 = (np.asarray(Wq, np.float32) * SCALE).astype(np.float16)
    wk16 = np.asarray(Wk, np.float32).astype(np.float16)
    wv16 = np.asarray(Wv, np.float32).astype(np.float16)

    def _slice_w(w):
        # [D, D] -> [t, ki, ko*128+c] so the per-t weight DMA is contiguous
        return np.ascontiguousarray(
            w.reshape(8, P, 8, P).transpose(2, 1, 0, 3).reshape(8, P, D)
        )

    wqs = _slice_w(wq16)
    wks = _slice_w(wk16)

    # fold mask in, pre-transpose to [b, h, k, q], exponentiate, cast fp16
    if np.any(mask):
        scores_add = prev + mask
    else:
        scores_add = prev
    em = np.exp(
        scores_add.transpose(0, 1, 3, 2).astype(np.float32) - EXP_SHIFT
    ).astype(np.float16)
    hT = np.ascontiguousarray(hs.transpose(0, 2, 1)).astype(np.float16)

    in_maps = []
    for b in range(N_CORES):
        m = {
            "hiddenT": np.ascontiguousarray(hT[b]),
            "wqs": wqs,
            "wks": wks,
            "wv": wv16,
            "em": np.ascontiguousarray(em[b]),
        }
        if use_bias:
            m["bq"] = (np.asarray(bq, np.float32) * SCALE).astype(np.float16)[None, :]
            m["bk"] = np.asarray(bk, np.float32).astype(np.float16)[None, :]
            m["bv"] = np.asarray(bv, np.float32).astype(np.float16)[None, :]
        in_maps.append(m)
    return in_maps


def kernel(hidden_states, attn_mask, prev_attn_weights, Wq, bq, Wk, bk, Wv, bv):
    from concourse.bass_utils import run_bass_kernel_spmd

    use_bias = bool(np.any(bq) or np.any(bk) or np.any(bv))
    nc = _get_compiled(use_bias)
    in_maps = _prepare_in_maps(
        hidden_states, attn_mask, prev_attn_weights, Wq, bq, Wk, bk, Wv, bv, use_bias
    )
    res = run_bass_kernel_spmd(nc, in_maps, core_ids=list(range(N_CORES)))
    return np.stack([res.results[b]["out"] for b in range(N_CORES)]).astype(np.float32)


# revision 39
# speedup vs baseline: 1.2898x; 1.2898x over previous
"""RealFormer-style MultiHeadAttention on 8 Trainium2 NeuronCores.

Reference computation (B=8, S=1024, D=1024, H=16, HD=64):
    q = split_heads(hidden @ Wq + bq); k = ...; v = ...
    scores = (q @ k^T) * HD**-0.5 + attn_mask + prev_attn_weights
    out    = merge_heads(softmax(scores) @ v)

Sharding: pure data-parallel over batch — one batch element per core,
no collectives.

Per-core kernel design (all matmul operands fp16, accumulation fp32):
  * Host folds SCALE into Wq, pre-transposes hidden, and ships
    E = exp(prevT + maskT - 4) in fp16 — turning the additive RealFormer
    residual into a multiplicative factor on the softmax numerator:
    exp(qk + prev - 8) = exp(qk - 4) * E.  This removes the PE inject
    matmul entirely; the multiply runs on the otherwise-idle DVE.
  * The Activation engine (exp over all S*S*H scores, ~133us busy) must
    never stall PE and must start early.  The schedule therefore begins
    scoring head 0 as soon as q/k block 0 is projected (~14us in), and
    all remaining projection work (v, q/k blocks t+1) is spread as PE
    "fill" between score tiles so PE's per-tile cadence stays above the
    exp() drain rate.  ctx for head h-2 is also interleaved per tile.
  * Per head: scoresT[k,q] = kT^T @ qT into PSUM; ScalarE writes
    exp(scoresT - 4) to fp16 SBUF; DVE multiplies by E in place.  The
    constant shift cancels in the normalization (row sums come free via
    a ones column in vx), so no row-max pass is needed.
  * ctxT[65, q] = vx^T @ probsT accumulated over k; DMA-xbar transpose
    to [q, 65]; VectorE reciprocal + per-partition scale; per-head
    output DMA so writeback overlaps compute.
"""

import sys

if "/opt/trn_rl_repo" not in sys.path:
    sys.path.insert(0, "/opt/trn_rl_repo")

import numpy as np

B, S, D, H = 8, 1024, 1024, 16
HD = D // H
SCALE = HD**-0.5
P = 128
N_CORES = 8
EXP_SHIFT = 4.0

_compiled = {}


def _build(use_bias: bool, reps: int = 1):
    import concourse.bacc as bacc
    import concourse.mybir as mybir
    import concourse.tile as tile

    f16 = mybir.dt.float16
    f32 = mybir.dt.float32
    Exp = mybir.ActivationFunctionType.Exp

    nc = bacc.Bacc("TRN2", target_bir_lowering=False, debug=False)

    hT_d = nc.dram_tensor("hiddenT", (D, S), f16, kind="ExternalInput").ap()
    # wq/wk are shipped pre-sliced by output-dim block t so each per-t
    # weight DMA is fully contiguous: [t, ki, ko*128+c]
    wqs_d = {
        name: nc.dram_tensor(name, (8, P, D), f16, kind="ExternalInput").ap()
        for name in ("wqs", "wks")
    }
    wv_d = nc.dram_tensor("wv", (D, D), f16, kind="ExternalInput").ap()
    em_d = nc.dram_tensor("em", (H, S, S), f16, kind="ExternalInput").ap()
    b_d = {}
    if use_bias:
        b_d = {
            name: nc.dram_tensor(name, (1, D), f16, kind="ExternalInput").ap()
            for name in ("bq", "bk", "bv")
        }
    out_d = nc.dram_tensor("out", (S, D), f32, kind="ExternalOutput").ap()
    out_v = out_d.rearrange("(qo qi) d -> qi qo d", qi=P)

    with tile.TileContext(nc) as tc:
        with (
            tc.tile_pool(name="big", bufs=1) as big,
            tc.tile_pool(name="wqk", bufs=4) as wqk_pool,
            tc.tile_pool(name="wv", bufs=8) as wv_pool,
            tc.tile_pool(name="ppool", bufs=6) as ppool,
            tc.tile_pool(name="probs", bufs=3) as probs_pool,
            tc.tile_pool(name="small", bufs=3) as small,
            tc.tile_pool(name="outp", bufs=3) as outp,
            tc.tile_pool(name="const", bufs=1) as const_pool,
            tc.tile_pool(name="ps_sc", bufs=2, space="PSUM") as ps_sc,
            tc.tile_pool(name="ps_ctx", bufs=1, space="PSUM") as ps_ctx,
            tc.tile_pool(name="ps_t", bufs=2, space="PSUM") as ps_t,
        ):
            for _rep in range(reps):
                neg_shift = const_pool.tile([P, 1], f32, name="negs")
                nc.any.memset(neg_shift, -EXP_SHIFT)
                ident = const_pool.tile([P, P], f16, name="ident")
                from concourse.masks import make_identity

                make_identity(nc, ident)
                if use_bias:
                    ones_row = const_pool.tile([1, 512], f16, name="ones")
                    nc.any.memset(ones_row, 1.0)
                    b_sb = {}
                    for name in ("bq", "bk", "bv"):
                        bt = const_pool.tile([1, D], f16, name=f"bsb_{name}")
                        nc.sync.dma_start(bt, b_d[name])
                        b_sb[name] = bt

                hidT = big.tile([P, 8, S], f16, tag="hidT")

                qT = big.tile([P, 8, S], f16, tag="qT")
                kT = big.tile([P, 8, S], f16, tag="kT")
                vx = big.tile([P, 8, H * 65], f16, tag="vx")
                vx_view = vx.rearrange("p t (h c) -> p t h c", c=65)
                nc.any.memset(vx_view[:, :, :, 64], 1.0)

                probsT_live = {}
                ctx_ps_live = {}
                fill_q = []

                def queue_proj(pname, t, dest):
                    # weight slice DMA issues now (prefetch); the 16 matmuls
                    # + PSUM->SBUF copy go into the PE fill queue
                    wsl = wqk_pool.tile(
                        [P, 8, P], f16, tag="wsl", name=f"wsl_{pname}{t}"
                    )
                    nc.sync.dma_start(
                        wsl,
                        wqs_d["w" + pname + "s"][t].rearrange(
                            "p (ko c) -> p ko c", c=P
                        ),
                    )
                    holder = {}
                    for half in range(2):
                        hs = slice(half * 512, half * 512 + 512)
                        for kt in range(8):

                            def mm(half=half, hs=hs, kt=kt):
                                if "pt" not in holder:
                                    holder["pt"] = ps_sc.tile(
                                        [P, S], f32, tag="pssc", name=f"pp_{pname}{t}"
                                    )
                                nc.tensor.matmul(
                                    holder["pt"][:, hs],
                                    lhsT=wsl[:, kt, :],
                                    rhs=hidT[:, kt, hs],
                                    start=(kt == 0),
                                    stop=(kt == 7 and not use_bias),
                                )

                            fill_q.append(mm)
                        if use_bias:

                            def mmb(hs=hs):
                                nc.tensor.matmul(
                                    holder["pt"][:, hs],
                                    lhsT=b_sb["b" + pname][:, t * P : (t + 1) * P],
                                    rhs=ones_row,
                                    start=False,
                                    stop=True,
                                )

                            fill_q.append(mmb)

                    def cp():
                        nc.vector.tensor_copy(dest[:, t, :], holder["pt"])

                    fill_q.append(cp)

                def queue_v():
                    wts = []
                    for kt in range(8):
                        wt = wv_pool.tile([P, D], f16, tag="w", name=f"w_v{kt}")
                        eng = nc.sync if kt % 2 == 0 else nc.gpsimd
                        eng.dma_start(wt, wv_d[kt * P : (kt + 1) * P, :])
                        wts.append(wt)
                    for pt_i in range(8):
                        holder = {}
                        for half in range(2):
                            hs = slice(half * 512, half * 512 + 512)
                            for dt in range(8):

                                def mm(pt_i=pt_i, hs=hs, dt=dt, holder=holder):
                                    if "pv" not in holder:
                                        holder["pv"] = ps_sc.tile(
                                            [P, D], f32, tag="pssc", name=f"pv_{pt_i}"
                                        )
                                    nc.tensor.matmul(
                                        holder["pv"][:, hs],
                                        lhsT=hidT[:, dt, pt_i * P : (pt_i + 1) * P],
                                        rhs=wts[dt][:, hs],
                                        start=(dt == 0),
                                        stop=(dt == 7 and not use_bias),
                                    )

                                fill_q.append(mm)
                            if use_bias:

                                def mmb(hs=hs, holder=holder):
                                    nc.tensor.matmul(
                                        holder["pv"][:, hs],
                                        lhsT=ones_row[:, :P],
                                        rhs=b_sb["bv"][:, hs],
                                        start=False,
                                        stop=True,
                                    )

                                fill_q.append(mmb)

                        def cp(pt_i=pt_i, holder=holder):
                            nc.vector.tensor_copy(
                                vx_view[:, pt_i, :, 0:64],
                                holder["pv"].rearrange("p (h e) -> p h e", e=64),
                            )

                        fill_q.append(cp)

                def emit_ctx_mm(hc, idx):
                    half, ktc = idx // 8, idx % 8
                    hs = slice(half * 512, half * 512 + 512)
                    nc.tensor.matmul(
                        ctx_ps_live[hc][:, hs],
                        lhsT=vx[:, ktc, hc * 65 : (hc + 1) * 65],
                        rhs=probsT_live[hc][:, ktc, hs],
                        start=(ktc == 0),
                        stop=(ktc == 7),
                    )

                def emit_ctx_tail(hc):
                    probsT_live.pop(hc)
                    pc = ctx_ps_live.pop(hc)
                    ctxT_sb = small.tile([65, S], f16, tag="ctxT", name=f"ctxT_{hc}")
                    nc.vector.tensor_copy(ctxT_sb, pc)
                    oh = outp.tile([P, 8, 64], f32, tag="outh", name=f"out_{hc}")
                    for qt in range(8):
                        # PE transpose into the ctx PSUM pool (pc is already
                        # freed by the copy above, so rotation is clean)
                        ptt = ps_t.tile([P, 65], f16, tag="pst", name=f"pt_{hc}_{qt}")
                        nc.tensor.matmul(
                            ptt,
                            lhsT=ctxT_sb[:, qt * P : (qt + 1) * P],
                            rhs=ident[0:65, 0:65],
                            is_transpose=True,
                        )
                        rc = small.tile([P, 1], f32, tag="recip", name=f"rc_{hc}_{qt}")
                        nc.vector.reciprocal(rc, ptt[:, 64:65])
                        nc.vector.tensor_scalar_mul(oh[:, qt, :], ptt[:, 0:64], rc)
                    nc.sync.dma_start(out_v[:, :, hc * 64 : (hc + 1) * 64], oh)

                def emit_head(h, hc, hc2=None, tail2=False, self_ctx=False, mult_chunk=4):
                    # scores for head h; ctx matmuls for head hc (=h-2) and
                    # optionally hc2 interleaved; PE fill (projection) work
                    # drained between score tiles.  self_ctx (last head)
                    # interleaves this head's own ctx half-0, lagged 4 slots
                    # behind the per-tile E-multiply.
                    r, t = h % 2, h // 2
                    rs = slice(r * 64, (r + 1) * 64)
                    ev_ap = em_d[h].rearrange("(ko ki) q -> ki ko q", ki=P)
                    em_sb = []
                    for j in range(2):
                        ej = ppool.tile([P, 4, S], f16, tag="prev", name=f"em_{h}_{j}")
                        eng = nc.sync if j == 0 else nc.gpsimd
                        eng.dma_start(ej, ev_ap[:, j * 4 : (j + 1) * 4, :])
                        em_sb.append(ej)

                    probsT = probs_pool.tile(
                        [P, 8, S], f16, tag="probsT", name=f"probsT_{h}"
                    )
                    probsT_live[h] = probsT
                    if hc is not None:
                        ctx_ps_live[hc] = ps_ctx.tile(
                            [65, S], f32, tag="psc", name=f"ps_c_{hc}"
                        )
                    if hc2 is not None:
                        # second ctx stream's PSUM comes from the score pool
                        # (the dedicated ctx slot is held by hc)
                        ctx_ps_live[hc2] = ps_sc.tile(
                            [65, S], f32, tag="pssc", name=f"ps_c_{hc2}"
                        )
                    n_kt = 8
                    for kt in range(n_kt):
                        ks = slice(kt * P, (kt + 1) * P)
                        ps = ps_sc.tile([P, S], f32, tag="pssc", name=f"ps_s_{h}_{kt}")
                        for half in range(2):
                            hs = slice(half * 512, half * 512 + 512)
                            nc.tensor.matmul(
                                ps[:, hs],
                                lhsT=kT[rs, t, ks],
                                rhs=qT[rs, t, hs],
                                start=True,
                                stop=True,
                            )
                        nc.scalar.activation(
                            probsT[:, kt, :], ps[:], Exp, bias=neg_shift
                        )
                        if kt % mult_chunk == mult_chunk - 1:
                            j0 = kt + 1 - mult_chunk
                            cv = probsT[:, j0 : kt + 1, :]
                            ej = em_sb[j0 // 4]
                            nc.vector.tensor_mul(
                                cv, cv, ej[:, j0 % 4 : j0 % 4 + mult_chunk, :]
                            )
                        if hc is not None:
                            emit_ctx_mm(hc, 2 * kt)
                            emit_ctx_mm(hc, 2 * kt + 1)
                        if hc2 is not None and not self_ctx:
                            emit_ctx_mm(hc2, 2 * kt)
                            emit_ctx_mm(hc2, 2 * kt + 1)
                        if self_ctx and kt >= 4:
                            emit_ctx_mm(h, 2 * (kt - 4))
                            emit_ctx_mm(h, 2 * (kt - 4) + 1)
                        # drain PE fill work (projections) for this slot
                        rem = len(fill_q)
                        if rem:
                            rate = -(-rem // (n_kt - kt))
                            rate = min(rate, 11)
                            for _ in range(min(rate, rem)):
                                fill_q.pop(0)()
                    if hc is not None:
                        emit_ctx_tail(hc)
                    if tail2 and hc2 is not None:
                        emit_ctx_tail(hc2)

                # --- schedule ---
                # q0/k0 projected bulk (PE's first work after hidT lands);
                # their weight DMAs are issued before the big hidT transfer
                queue_proj("q", 0, qT)
                hT_v = hT_d.rearrange("(do di) s -> di do s", di=P)
                nc.sync.dma_start(hidT[:, :, 0:512], hT_v[:, :, 0:512])
                nc.sync.dma_start(hidT[:, :, 512:S], hT_v[:, :, 512:S])
                queue_proj("k", 0, kT)
                while fill_q:
                    fill_q.pop(0)()
                queue_v()
                queue_proj("q", 1, qT)
                queue_proj("k", 1, kT)
                for h in range(16):
                    t = h // 2
                    if h >= 2 and h % 2 == 0 and t + 1 <= 7:
                        queue_proj("q", t + 1, qT)
                    if h >= 2 and h % 2 == 1 and t + 1 <= 7:
                        queue_proj("k", t + 1, kT)
                    emit_head(h, h - 2 if h >= 2 else None,
                              mult_chunk=2 if h == 15 else 4)
                ctx_ps_live[14] = ps_ctx.tile([65, S], f32, tag="psc", name="ps_c_14")
                for idx in range(16):
                    emit_ctx_mm(14, idx)
                emit_ctx_tail(14)
                ctx_ps_live[15] = ps_ctx.tile([65, S], f32, tag="psc", name="ps_c_15")
                for idx in range(8):
                    emit_ctx_mm(15, idx)
                # ctx15: copy finished half-0 while half-1 matmuls run
                pc15 = ctx_ps_live[15]
                ctxT_sb = small.tile([65, S], f16, tag="ctxT", name="ctxT_15")
                nc.vector.tensor_copy(ctxT_sb[:, 0:512], pc15[:, 0:512])
                for idx in range(8, 16):
                    emit_ctx_mm(15, idx)
                nc.vector.tensor_copy(ctxT_sb[:, 512:S], pc15[:, 512:S])
                oh = outp.tile([P, 8, 64], f32, tag="outh", name="out_15")
                for qt in range(8):
                    ptt = ps_t.tile([P, 65], f16, tag="pst", name=f"pt_15_{qt}")
                    nc.tensor.matmul(
                        ptt,
                        lhsT=ctxT_sb[:, qt * P : (qt + 1) * P],
                        rhs=ident[0:65, 0:65],
                        is_transpose=True,
                    )
                    rc = small.tile([P, 1], f32, tag="recip", name=f"rc_15_{qt}")
                    nc.vector.reciprocal(rc, ptt[:, 64:65])
                    nc.vector.tensor_scalar_mul(oh[:, qt, :], ptt[:, 0:64], rc)
                nc.sync.dma_start(out_v[:, :, 15 * 64 : 16 * 64], oh)
                probsT_live.pop(15)
                ctx_ps_live.pop(15)

    nc.compile()
    return nc


def _get_compiled(use_bias: bool, reps: int = 1):
    key = (use_bias, reps)
    if key not in _compiled:
        _compiled[key] = _build(use_bias, reps)
    return _compiled[key]


def _prepare_in_maps(
    hidden_states, attn_mask, prev_attn_weights, Wq, bq, Wk, bk, Wv, bv, use_bias
):
    hs = np.asarray(hidden_states, np.float32)
    mask = np.asarray(attn_mask, np.float32)
    prev = np.asarray(prev_attn_weights, np.float32)

    wq16 = (np.asarray(Wq, np.float32) * SCALE).astype(np.float16)
    wk16 = np.asarray(Wk, np.float32).astype(np.float16)
    wv16 = np.asarray(Wv, np.float32).astype(np.float16)

    def _slice_w(w):
        # [D, D] -> [t, ki, ko*128+c] so the per-t weight DMA is contiguous
        return np.ascontiguousarray(
            w.reshape(8, P, 8, P).transpose(2, 1, 0, 3).reshape(8, P, D)
        )

    wqs = _slice_w(wq16)
    wks = _slice_w(wk16)

    # fold mask in, pre-transpose to [b, h, k, q], exponentiate, cast fp16
    if np.any(mask):
        scores_add = prev + mask
    else:
        scores_add = prev
    em = np.exp(
        scores_add.transpose(0, 1, 3, 2).astype(np.float32) - EXP_SHIFT
    ).astype(np.float16)
    hT = np.ascontiguousarray(hs.transpose(0, 2, 1)).astype(np.float16)

    in_maps = []
    for b in range(N_CORES):
        m = {
            "hiddenT": np.ascontiguousarray(hT[b]),
            "wqs": wqs,
            "wks": wks,
            "wv": wv16,
            "em": np.ascontiguousarray(em[b]),
        }
        if use_bias:
            m["bq"] = (np.asarray(bq, np.float32) * SCALE).astype(np.float16)[None, :]
            m["bk"] = np.asarray(bk, np.float32).astype(np.float16)[None, :]
            m["bv"] = np.asarray(bv, np.float32).astype(np.float16)[None, :]
        in_maps.append(m)
    return in_maps


def kernel(hidden_states, attn_mask, prev_attn_weights, Wq, bq, Wk, bk, Wv, bv):
    from concourse.bass_utils import run_bass_kernel_spmd

    use_bias = bool(np.any(bq) or np.any(bk) or np.any(bv))
    nc = _get_compiled(use_bias)
    in_maps = _prepare_in_maps(
        hidden_states, attn_mask, prev_attn_weights, Wq, bq, Wk, bk, Wv, bv, use_bias
    )
    res = run_bass_kernel_spmd(nc, in_maps, core_ids=list(range(N_CORES)))
    return np.stack([res.results[b]["out"] for b in range(N_CORES)]).astype(np.float32)


# revision 44
# speedup vs baseline: 1.4242x; 1.1042x over previous
"""RealFormer-style MultiHeadAttention on 8 Trainium2 NeuronCores.

Reference computation (B=8, S=1024, D=1024, H=16, HD=64):
    q = split_heads(hidden @ Wq + bq); k = ...; v = ...
    scores = (q @ k^T) * HD**-0.5 + attn_mask + prev_attn_weights
    out    = merge_heads(softmax(scores) @ v)

Sharding: pure data-parallel over batch — one batch element per core,
no collectives.

Per-core kernel design (all matmul operands fp16, accumulation fp32):
  * Host folds SCALE into Wq, pre-transposes hidden, and ships
    E = exp(prevT + maskT - 4) in fp16 — turning the additive RealFormer
    residual into a multiplicative factor on the softmax numerator:
    exp(qk + prev - 8) = exp(qk - 4) * E.  This removes the PE inject
    matmul entirely; the multiply runs on the otherwise-idle DVE.
  * The Activation engine (exp over all S*S*H scores, ~133us busy) must
    never stall PE and must start early.  The schedule therefore begins
    scoring head 0 as soon as q/k block 0 is projected (~14us in), and
    all remaining projection work (v, q/k blocks t+1) is spread as PE
    "fill" between score tiles so PE's per-tile cadence stays above the
    exp() drain rate.  ctx for head h-2 is also interleaved per tile.
  * Per head: scoresT[k,q] = kT^T @ qT into PSUM; ScalarE writes
    exp(scoresT - 4) to fp16 SBUF; DVE multiplies by E in place.  The
    constant shift cancels in the normalization (row sums come free via
    a ones column in vx), so no row-max pass is needed.
  * ctxT[65, q] = vx^T @ probsT accumulated over k; DMA-xbar transpose
    to [q, 65]; VectorE reciprocal + per-partition scale; per-head
    output DMA so writeback overlaps compute.
"""

import sys

if "/opt/trn_rl_repo" not in sys.path:
    sys.path.insert(0, "/opt/trn_rl_repo")

import numpy as np

B, S, D, H = 8, 1024, 1024, 16
HD = D // H
SCALE = HD**-0.5
P = 128
N_CORES = 8
EXP_SHIFT = 4.0

_compiled = {}


def _build(use_bias: bool, reps: int = 1):
    import concourse.bacc as bacc
    import concourse.mybir as mybir
    import concourse.tile as tile

    f16 = mybir.dt.float16
    f32 = mybir.dt.float32
    Exp = mybir.ActivationFunctionType.Exp

    nc = bacc.Bacc("TRN2", target_bir_lowering=False, debug=False)

    hT_d = nc.dram_tensor("hiddenT", (D, S), f16, kind="ExternalInput").ap()
    # wq/wk are shipped pre-sliced by output-dim block t so each per-t
    # weight DMA is fully contiguous: [t, ki, ko*128+c]
    wqs_d = {
        name: nc.dram_tensor(name, (8, P, D), f16, kind="ExternalInput").ap()
        for name in ("wqs", "wks")
    }
    wv_d = nc.dram_tensor("wv", (D, D), f16, kind="ExternalInput").ap()
    em_d = nc.dram_tensor("em", (H, S, S), f16, kind="ExternalInput").ap()
    b_d = {}
    if use_bias:
        b_d = {
            name: nc.dram_tensor(name, (1, D), f16, kind="ExternalInput").ap()
            for name in ("bq", "bk", "bv")
        }
    out_d = nc.dram_tensor("out", (S, D), f32, kind="ExternalOutput").ap()
    out_v = out_d.rearrange("(qo qi) d -> qi qo d", qi=P)

    with tile.TileContext(nc) as tc:
        with (
            tc.tile_pool(name="big", bufs=1) as big,
            tc.tile_pool(name="wqk", bufs=4) as wqk_pool,
            tc.tile_pool(name="wv", bufs=8) as wv_pool,
            tc.tile_pool(name="ppool", bufs=6) as ppool,
            tc.tile_pool(name="probs", bufs=3) as probs_pool,
            tc.tile_pool(name="small", bufs=3) as small,
            tc.tile_pool(name="outp", bufs=3) as outp,
            tc.tile_pool(name="const", bufs=1) as const_pool,
            tc.tile_pool(name="ps_sc", bufs=2, space="PSUM") as ps_sc,
            tc.tile_pool(name="ps_ctx", bufs=1, space="PSUM") as ps_ctx,
            tc.tile_pool(name="ps_t", bufs=2, space="PSUM") as ps_t,
        ):
            for _rep in range(reps):
                neg_shift = const_pool.tile([P, 1], f32, name="negs")
                nc.any.memset(neg_shift, -EXP_SHIFT)
                ident = const_pool.tile([P, P], f16, name="ident")
                from concourse.masks import make_identity

                make_identity(nc, ident)
                if use_bias:
                    ones_row = const_pool.tile([1, 512], f16, name="ones")
                    nc.any.memset(ones_row, 1.0)
                    b_sb = {}
                    for name in ("bq", "bk", "bv"):
                        bt = const_pool.tile([1, D], f16, name=f"bsb_{name}")
                        nc.sync.dma_start(bt, b_d[name])
                        b_sb[name] = bt

                hidT = big.tile([P, 8, S], f16, tag="hidT")

                qT = big.tile([P, 8, S], f16, tag="qT")
                kT = big.tile([P, 8, S], f16, tag="kT")
                vx = big.tile([P, 8, H * 65], f16, tag="vx")
                vx_view = vx.rearrange("p t (h c) -> p t h c", c=65)
                nc.any.memset(vx_view[:, :, :, 64], 1.0)

                probsT_live = {}
                ctx_ps_live = {}
                fill_q = []

                def queue_proj(pname, t, dest):
                    # weight slice DMA issues now (prefetch); the 16 matmuls
                    # + PSUM->SBUF copy go into the PE fill queue
                    wsl = wqk_pool.tile(
                        [P, 8, P], f16, tag="wsl", name=f"wsl_{pname}{t}"
                    )
                    nc.sync.dma_start(
                        wsl,
                        wqs_d["w" + pname + "s"][t].rearrange(
                            "p (ko c) -> p ko c", c=P
                        ),
                    )
                    holder = {}
                    for half in range(2):
                        hs = slice(half * 512, half * 512 + 512)
                        for kt in range(8):

                            def mm(half=half, hs=hs, kt=kt):
                                if "pt" not in holder:
                                    holder["pt"] = ps_sc.tile(
                                        [P, S], f32, tag="pssc", name=f"pp_{pname}{t}"
                                    )
                                nc.tensor.matmul(
                                    holder["pt"][:, hs],
                                    lhsT=wsl[:, kt, :],
                                    rhs=hidT[:, kt, hs],
                                    start=(kt == 0),
                                    stop=(kt == 7 and not use_bias),
                                )

                            fill_q.append(mm)
                        if use_bias:

                            def mmb(hs=hs):
                                nc.tensor.matmul(
                                    holder["pt"][:, hs],
                                    lhsT=b_sb["b" + pname][:, t * P : (t + 1) * P],
                                    rhs=ones_row,
                                    start=False,
                                    stop=True,
                                )

                            fill_q.append(mmb)

                    def cp():
                        nc.vector.tensor_copy(dest[:, t, :], holder["pt"])

                    fill_q.append(cp)

                def queue_v():
                    wts = []
                    for kt in range(8):
                        wt = wv_pool.tile([P, D], f16, tag="w", name=f"w_v{kt}")
                        nc.sync.dma_start(wt, wv_d[kt * P : (kt + 1) * P, :])
                        wts.append(wt)
                    for pt_i in range(8):
                        holder = {}
                        for half in range(2):
                            hs = slice(half * 512, half * 512 + 512)
                            for dt in range(8):

                                def mm(pt_i=pt_i, hs=hs, dt=dt, holder=holder):
                                    if "pv" not in holder:
                                        holder["pv"] = ps_sc.tile(
                                            [P, D], f32, tag="pssc", name=f"pv_{pt_i}"
                                        )
                                    nc.tensor.matmul(
                                        holder["pv"][:, hs],
                                        lhsT=hidT[:, dt, pt_i * P : (pt_i + 1) * P],
                                        rhs=wts[dt][:, hs],
                                        start=(dt == 0),
                                        stop=(dt == 7 and not use_bias),
                                    )

                                fill_q.append(mm)
                            if use_bias:

                                def mmb(hs=hs, holder=holder):
                                    nc.tensor.matmul(
                                        holder["pv"][:, hs],
                                        lhsT=ones_row[:, :P],
                                        rhs=b_sb["bv"][:, hs],
                                        start=False,
                                        stop=True,
                                    )

                                fill_q.append(mmb)

                        def cp(pt_i=pt_i, holder=holder):
                            nc.vector.tensor_copy(
                                vx_view[:, pt_i, :, 0:64],
                                holder["pv"].rearrange("p (h e) -> p h e", e=64),
                            )

                        fill_q.append(cp)

                def emit_ctx_mm(hc, idx):
                    half, ktc = idx // 8, idx % 8
                    hs = slice(half * 512, half * 512 + 512)
                    nc.tensor.matmul(
                        ctx_ps_live[hc][:, hs],
                        lhsT=vx[:, ktc, hc * 65 : (hc + 1) * 65],
                        rhs=probsT_live[hc][:, ktc, hs],
                        start=(ktc == 0),
                        stop=(ktc == 7),
                    )

                def emit_ctx_tail(hc):
                    probsT_live.pop(hc)
                    pc = ctx_ps_live.pop(hc)
                    ctxT_sb = small.tile([65, S], f16, tag="ctxT", name=f"ctxT_{hc}")
                    nc.vector.tensor_copy(ctxT_sb, pc)
                    oh = outp.tile([P, 8, 64], f32, tag="outh", name=f"out_{hc}")
                    for qt in range(8):
                        # PE transpose into the ctx PSUM pool (pc is already
                        # freed by the copy above, so rotation is clean)
                        ptt = ps_t.tile([P, 65], f16, tag="pst", name=f"pt_{hc}_{qt}")
                        nc.tensor.matmul(
                            ptt,
                            lhsT=ctxT_sb[:, qt * P : (qt + 1) * P],
                            rhs=ident[0:65, 0:65],
                            is_transpose=True,
                        )
                        rc = small.tile([P, 1], f32, tag="recip", name=f"rc_{hc}_{qt}")
                        nc.vector.reciprocal(rc, ptt[:, 64:65])
                        nc.vector.tensor_scalar_mul(oh[:, qt, :], ptt[:, 0:64], rc)
                    nc.sync.dma_start(out_v[:, :, hc * 64 : (hc + 1) * 64], oh)

                def emit_head(h, hc, hc2=None, tail2=False, self_ctx=False, mult_chunk=4):
                    # scores for head h; ctx matmuls for head hc (=h-2) and
                    # optionally hc2 interleaved; PE fill (projection) work
                    # drained between score tiles.  self_ctx (last head)
                    # interleaves this head's own ctx half-0, lagged 4 slots
                    # behind the per-tile E-multiply.
                    r, t = h % 2, h // 2
                    rs = slice(r * 64, (r + 1) * 64)
                    ev_ap = em_d[h].rearrange("(ko ki) q -> ki ko q", ki=P)
                    em_sb = []
                    for j in range(2):
                        ej = ppool.tile([P, 4, S], f16, tag="prev", name=f"em_{h}_{j}")
                        nc.sync.dma_start(ej, ev_ap[:, j * 4 : (j + 1) * 4, :])
                        em_sb.append(ej)

                    probsT = probs_pool.tile(
                        [P, 8, S], f16, tag="probsT", name=f"probsT_{h}"
                    )
                    probsT_live[h] = probsT
                    if hc is not None:
                        ctx_ps_live[hc] = ps_ctx.tile(
                            [65, S], f32, tag="psc", name=f"ps_c_{hc}"
                        )
                    if hc2 is not None:
                        # second ctx stream's PSUM comes from the score pool
                        # (the dedicated ctx slot is held by hc)
                        ctx_ps_live[hc2] = ps_sc.tile(
                            [65, S], f32, tag="pssc", name=f"ps_c_{hc2}"
                        )
                    n_kt = 8
                    for kt in range(n_kt):
                        ks = slice(kt * P, (kt + 1) * P)
                        ps = ps_sc.tile([P, S], f32, tag="pssc", name=f"ps_s_{h}_{kt}")
                        for half in range(2):
                            hs = slice(half * 512, half * 512 + 512)
                            nc.tensor.matmul(
                                ps[:, hs],
                                lhsT=kT[rs, t, ks],
                                rhs=qT[rs, t, hs],
                                start=True,
                                stop=True,
                            )
                        nc.scalar.activation(
                            probsT[:, kt, :], ps[:], Exp, bias=neg_shift
                        )
                        if kt % mult_chunk == mult_chunk - 1:
                            j0 = kt + 1 - mult_chunk
                            cv = probsT[:, j0 : kt + 1, :]
                            ej = em_sb[j0 // 4]
                            nc.vector.tensor_mul(
                                cv, cv, ej[:, j0 % 4 : j0 % 4 + mult_chunk, :]
                            )
                        if hc is not None:
                            emit_ctx_mm(hc, 2 * kt)
                            emit_ctx_mm(hc, 2 * kt + 1)
                        if hc2 is not None and not self_ctx:
                            emit_ctx_mm(hc2, 2 * kt)
                            emit_ctx_mm(hc2, 2 * kt + 1)
                        if self_ctx and kt >= 4:
                            emit_ctx_mm(h, 2 * (kt - 4))
                            emit_ctx_mm(h, 2 * (kt - 4) + 1)
                        # drain PE fill work (projections) for this slot
                        rem = len(fill_q)
                        if rem:
                            rate = -(-rem // (n_kt - kt))
                            rate = min(rate, 11)
                            for _ in range(min(rate, rem)):
                                fill_q.pop(0)()
                    if hc is not None:
                        emit_ctx_tail(hc)
                    if tail2 and hc2 is not None:
                        emit_ctx_tail(hc2)

                # --- schedule ---
                # q0/k0 projected bulk (PE's first work after hidT lands);
                # their weight DMAs are issued before the big hidT transfer
                queue_proj("q", 0, qT)
                hT_v = hT_d.rearrange("(do di) s -> di do s", di=P)
                nc.sync.dma_start(hidT[:, :, 0:512], hT_v[:, :, 0:512])
                nc.sync.dma_start(hidT[:, :, 512:S], hT_v[:, :, 512:S])
                queue_proj("k", 0, kT)
                while fill_q:
                    fill_q.pop(0)()
                queue_v()
                queue_proj("q", 1, qT)
                queue_proj("k", 1, kT)
                for h in range(16):
                    t = h // 2
                    if h >= 2 and h % 2 == 0 and t + 1 <= 7:
                        queue_proj("q", t + 1, qT)
                    if h >= 2 and h % 2 == 1 and t + 1 <= 7:
                        queue_proj("k", t + 1, kT)
                    emit_head(h, h - 2 if h >= 2 else None,
                              mult_chunk=2 if h == 15 else 4)
                ctx_ps_live[14] = ps_ctx.tile([65, S], f32, tag="psc", name="ps_c_14")
                for idx in range(16):
                    emit_ctx_mm(14, idx)
                emit_ctx_tail(14)
                ctx_ps_live[15] = ps_ctx.tile([65, S], f32, tag="psc", name="ps_c_15")
                for idx in range(8):
                    emit_ctx_mm(15, idx)
                # ctx15: copy finished half-0 while half-1 matmuls run
                pc15 = ctx_ps_live[15]
                ctxT_sb = small.tile([65, S], f16, tag="ctxT", name="ctxT_15")
                nc.vector.tensor_copy(ctxT_sb[:, 0:512], pc15[:, 0:512])
                for idx in range(8, 16):
                    emit_ctx_mm(15, idx)
                nc.vector.tensor_copy(ctxT_sb[:, 512:S], pc15[:, 512:S])
                oh = outp.tile([P, 8, 64], f32, tag="outh", name="out_15")
                for qt in range(8):
                    ptt = ps_t.tile([P, 65], f16, tag="pst", name=f"pt_15_{qt}")
                    nc.tensor.matmul(
                        ptt,
                        lhsT=ctxT_sb[:, qt * P : (qt + 1) * P],
                        rhs=ident[0:65, 0:65],
                        is_transpose=True,
                    )
                    rc = small.tile([P, 1], f32, tag="recip", name=f"rc_15_{qt}")
                    nc.vector.reciprocal(rc, ptt[:, 64:65])
                    nc.vector.tensor_scalar_mul(oh[:, qt, :], ptt[:, 0:64], rc)
                nc.sync.dma_start(out_v[:, :, 15 * 64 : 16 * 64], oh)
                probsT_live.pop(15)
                ctx_ps_live.pop(15)

    nc.compile()
    return nc


def _get_compiled(use_bias: bool, reps: int = 1):
    key = (use_bias, reps)
    if key not in _compiled:
        _compiled[key] = _build(use_bias, reps)
    return _compiled[key]


def _prepare_in_maps(
    hidden_states, attn_mask, prev_attn_weights, Wq, bq, Wk, bk, Wv, bv, use_bias
):
    hs = np.asarray(hidden_states, np.float32)
    mask = np.asarray(attn_mask, np.float32)
    prev = np.asarray(prev_attn_weights, np.float32)

    wq16 = (np.asarray(Wq, np.float32) * SCALE).astype(np.float16)
    wk16 = np.asarray(Wk, np.float32).astype(np.float16)
    wv16 = np.asarray(Wv, np.float32).astype(np.float16)

    def _slice_w(w):
        # [D, D] -> [t, ki, ko*128+c] so the per-t weight DMA is contiguous
        return np.ascontiguousarray(
            w.reshape(8, P, 8, P).transpose(2, 1, 0, 3).reshape(8, P, D)
        )

    wqs = _slice_w(wq16)
    wks = _slice_w(wk16)

    # fold mask in, pre-transpose to [b, h, k, q], exponentiate, cast fp16
    if np.any(mask):
        scores_add = prev + mask
    else:
        scores_add = prev
    em = np.exp(
        scores_add.transpose(0, 1, 3, 2).astype(np.float32) - EXP_SHIFT
    ).astype(np.float16)
    hT = np.ascontiguousarray(hs.transpose(0, 2, 1)).astype(np.float16)

    in_maps = []
    for b in range(N_CORES):
        m = {
            "hiddenT": np.ascontiguousarray(hT[b]),
            "wqs": wqs,
            "wks": wks,
            "wv": wv16,
            "em": np.ascontiguousarray(em[b]),
        }
        if use_bias:
            m["bq"] = (np.asarray(bq, np.float32) * SCALE).astype(np.float16)[None, :]
            m["bk"] = np.asarray(bk, np.float32).astype(np.float16)[None, :]
            m["bv"] = np.asarray(bv, np.float32).astype(np.float16)[None, :]
        in_maps.append(m)
    return in_maps


def kernel(hidden_states, attn_mask, prev_attn_weights, Wq, bq, Wk, bk, Wv, bv):
    from concourse.bass_utils import run_bass_kernel_spmd

    use_bias = bool(np.any(bq) or np.any(bk) or np.any(bv))
    nc = _get_compiled(use_bias)
    in_maps = _prepare_in_maps(
        hidden_states, attn_mask, prev_attn_weights, Wq, bq, Wk, bk, Wv, bv, use_bias
    )
    res = run_bass_kernel_spmd(nc, in_maps, core_ids=list(range(N_CORES)))
    return np.stack([res.results[b]["out"] for b in range(N_CORES)]).astype(np.float32)


# revision 46
# speedup vs baseline: 1.6482x; 1.1573x over previous
"""RealFormer-style MultiHeadAttention on 8 Trainium2 NeuronCores.

Reference computation (B=8, S=1024, D=1024, H=16, HD=64):
    q = split_heads(hidden @ Wq + bq); k = ...; v = ...
    scores = (q @ k^T) * HD**-0.5 + attn_mask + prev_attn_weights
    out    = merge_heads(softmax(scores) @ v)

Sharding: pure data-parallel over batch — one batch element per core,
no collectives.

Per-core kernel design (all matmul operands fp16, accumulation fp32):
  * Host folds SCALE into Wq, pre-transposes hidden, and ships
    E = exp(prevT + maskT - 4) in fp16 — turning the additive RealFormer
    residual into a multiplicative factor on the softmax numerator:
    exp(qk + prev - 8) = exp(qk - 4) * E.  This removes the PE inject
    matmul entirely; the multiply runs on the otherwise-idle DVE.
  * The Activation engine (exp over all S*S*H scores, ~133us busy) must
    never stall PE and must start early.  The schedule therefore begins
    scoring head 0 as soon as q/k block 0 is projected (~14us in), and
    all remaining projection work (v, q/k blocks t+1) is spread as PE
    "fill" between score tiles so PE's per-tile cadence stays above the
    exp() drain rate.  ctx for head h-2 is also interleaved per tile.
  * Per head: scoresT[k,q] = kT^T @ qT into PSUM; ScalarE writes
    exp(scoresT - 4) to fp16 SBUF; DVE multiplies by E in place.  The
    constant shift cancels in the normalization (row sums come free via
    a ones column in vx), so no row-max pass is needed.
  * ctxT[65, q] = vx^T @ probsT accumulated over k; DMA-xbar transpose
    to [q, 65]; VectorE reciprocal + per-partition scale; per-head
    output DMA so writeback overlaps compute.
"""

import sys

if "/opt/trn_rl_repo" not in sys.path:
    sys.path.insert(0, "/opt/trn_rl_repo")

import numpy as np

B, S, D, H = 8, 1024, 1024, 16
HD = D // H
SCALE = HD**-0.5
P = 128
N_CORES = 8
EXP_SHIFT = 4.0

_compiled = {}


def _build(use_bias: bool, reps: int = 1):
    import concourse.bacc as bacc
    import concourse.mybir as mybir
    import concourse.tile as tile

    f16 = mybir.dt.float16
    f32 = mybir.dt.float32
    Exp = mybir.ActivationFunctionType.Exp

    nc = bacc.Bacc("TRN2", target_bir_lowering=False, debug=False)

    hT_d = nc.dram_tensor("hiddenT", (D, S), f16, kind="ExternalInput").ap()
    # wq/wk are shipped pre-sliced by output-dim block t so each per-t
    # weight DMA is fully contiguous: [t, ki, ko*128+c]
    wqs_d = {
        name: nc.dram_tensor(name, (8, P, D), f16, kind="ExternalInput").ap()
        for name in ("wqs", "wks")
    }
    wv_d = nc.dram_tensor("wv", (D, D), f16, kind="ExternalInput").ap()
    em_d = nc.dram_tensor("em", (H, S, S), f16, kind="ExternalInput").ap()
    b_d = {}
    if use_bias:
        b_d = {
            name: nc.dram_tensor(name, (1, D), f16, kind="ExternalInput").ap()
            for name in ("bq", "bk", "bv")
        }
    out_d = nc.dram_tensor("out", (S, D), f32, kind="ExternalOutput").ap()
    out_v = out_d.rearrange("(qo qi) d -> qi qo d", qi=P)

    with tile.TileContext(nc) as tc:
        with (
            tc.tile_pool(name="big", bufs=1) as big,
            tc.tile_pool(name="wqk", bufs=4) as wqk_pool,
            tc.tile_pool(name="wv", bufs=8) as wv_pool,
            tc.tile_pool(name="ppool", bufs=6) as ppool,
            tc.tile_pool(name="probs", bufs=3) as probs_pool,
            tc.tile_pool(name="small", bufs=3) as small,
            tc.tile_pool(name="outp", bufs=3) as outp,
            tc.tile_pool(name="const", bufs=1) as const_pool,
            tc.tile_pool(name="ps_sc", bufs=2, space="PSUM") as ps_sc,
            tc.tile_pool(name="ps_ctx", bufs=1, space="PSUM") as ps_ctx,
            tc.tile_pool(name="ps_t", bufs=2, space="PSUM") as ps_t,
        ):
            for _rep in range(reps):
                neg_shift = const_pool.tile([P, 1], f32, name="negs")
                nc.any.memset(neg_shift, -EXP_SHIFT)
                ident = const_pool.tile([P, P], f16, name="ident")
                from concourse.masks import make_identity

                make_identity(nc, ident)
                if use_bias:
                    ones_row = const_pool.tile([1, 512], f16, name="ones")
                    nc.any.memset(ones_row, 1.0)
                    b_sb = {}
                    for name in ("bq", "bk", "bv"):
                        bt = const_pool.tile([1, D], f16, name=f"bsb_{name}")
                        nc.sync.dma_start(bt, b_d[name])
                        b_sb[name] = bt

                hidT = big.tile([P, 8, S], f16, tag="hidT")

                qT = big.tile([P, 8, S], f16, tag="qT")
                kT = big.tile([P, 8, S], f16, tag="kT")
                vx = big.tile([P, 8, H * 65], f16, tag="vx")
                vx_view = vx.rearrange("p t (h c) -> p t h c", c=65)
                nc.any.memset(vx_view[:, :, :, 64], 1.0)

                probsT_live = {}
                ctx_ps_live = {}
                fill_q = []

                def queue_proj(pname, t, dest):
                    # weight slice DMA issues now (prefetch); the 16 matmuls
                    # + PSUM->SBUF copy go into the PE fill queue
                    wsl = wqk_pool.tile(
                        [P, 8, P], f16, tag="wsl", name=f"wsl_{pname}{t}"
                    )
                    nc.sync.dma_start(
                        wsl,
                        wqs_d["w" + pname + "s"][t].rearrange(
                            "p (ko c) -> p ko c", c=P
                        ),
                    )
                    holder = {}
                    for half in range(2):
                        hs = slice(half * 512, half * 512 + 512)
                        for kt in range(8):

                            def mm(half=half, hs=hs, kt=kt):
                                if "pt" not in holder:
                                    holder["pt"] = ps_sc.tile(
                                        [P, S], f32, tag="pssc", name=f"pp_{pname}{t}"
                                    )
                                nc.tensor.matmul(
                                    holder["pt"][:, hs],
                                    lhsT=wsl[:, kt, :],
                                    rhs=hidT[:, kt, hs],
                                    start=(kt == 0),
                                    stop=(kt == 7 and not use_bias),
                                )

                            fill_q.append(mm)
                        if use_bias:

                            def mmb(hs=hs):
                                nc.tensor.matmul(
                                    holder["pt"][:, hs],
                                    lhsT=b_sb["b" + pname][:, t * P : (t + 1) * P],
                                    rhs=ones_row,
                                    start=False,
                                    stop=True,
                                )

                            fill_q.append(mmb)

                    def cp():
                        nc.vector.tensor_copy(dest[:, t, :], holder["pt"])

                    fill_q.append(cp)

                def queue_v():
                    wts = []
                    for kt in range(8):
                        wt = wv_pool.tile([P, D], f16, tag="w", name=f"w_v{kt}")
                        nc.sync.dma_start(wt, wv_d[kt * P : (kt + 1) * P, :])
                        wts.append(wt)
                    for pt_i in range(8):
                        holder = {}
                        for half in range(2):
                            hs = slice(half * 512, half * 512 + 512)
                            for dt in range(8):

                                def mm(pt_i=pt_i, hs=hs, dt=dt, holder=holder):
                                    if "pv" not in holder:
                                        holder["pv"] = ps_sc.tile(
                                            [P, D], f32, tag="pssc", name=f"pv_{pt_i}"
                                        )
                                    nc.tensor.matmul(
                                        holder["pv"][:, hs],
                                        lhsT=hidT[:, dt, pt_i * P : (pt_i + 1) * P],
                                        rhs=wts[dt][:, hs],
                                        start=(dt == 0),
                                        stop=(dt == 7 and not use_bias),
                                    )

                                fill_q.append(mm)
                            if use_bias:

                                def mmb(hs=hs, holder=holder):
                                    nc.tensor.matmul(
                                        holder["pv"][:, hs],
                                        lhsT=ones_row[:, :P],
                                        rhs=b_sb["bv"][:, hs],
                                        start=False,
                                        stop=True,
                                    )

                                fill_q.append(mmb)

                        def cp(pt_i=pt_i, holder=holder):
                            nc.vector.tensor_copy(
                                vx_view[:, pt_i, :, 0:64],
                                holder["pv"].rearrange("p (h e) -> p h e", e=64),
                            )

                        fill_q.append(cp)

                def emit_ctx_mm(hc, idx):
                    half, ktc = idx // 8, idx % 8
                    hs = slice(half * 512, half * 512 + 512)
                    nc.tensor.matmul(
                        ctx_ps_live[hc][:, hs],
                        lhsT=vx[:, ktc, hc * 65 : (hc + 1) * 65],
                        rhs=probsT_live[hc][:, ktc, hs],
                        start=(ktc == 0),
                        stop=(ktc == 7),
                    )

                def emit_ctx_tail(hc):
                    probsT_live.pop(hc)
                    pc = ctx_ps_live.pop(hc)
                    ctxT_sb = small.tile([65, S], f16, tag="ctxT", name=f"ctxT_{hc}")
                    nc.vector.tensor_copy(ctxT_sb, pc)
                    oh = outp.tile([P, 8, 64], f32, tag="outh", name=f"out_{hc}")
                    for qt in range(8):
                        # PE transpose into the ctx PSUM pool (pc is already
                        # freed by the copy above, so rotation is clean)
                        ptt = ps_t.tile([P, 65], f16, tag="pst", name=f"pt_{hc}_{qt}")
                        nc.tensor.matmul(
                            ptt,
                            lhsT=ctxT_sb[:, qt * P : (qt + 1) * P],
                            rhs=ident[0:65, 0:65],
                            is_transpose=True,
                        )
                        rc = small.tile([P, 1], f32, tag="recip", name=f"rc_{hc}_{qt}")
                        nc.vector.reciprocal(rc, ptt[:, 64:65])
                        nc.vector.tensor_scalar_mul(oh[:, qt, :], ptt[:, 0:64], rc)
                    nc.sync.dma_start(out_v[:, :, hc * 64 : (hc + 1) * 64], oh)

                def emit_head(h, hc, hc2=None, tail2=False, self_ctx=False, mult_chunk=4):
                    # scores for head h; ctx matmuls for head hc (=h-2) and
                    # optionally hc2 interleaved; PE fill (projection) work
                    # drained between score tiles.  self_ctx (last head)
                    # interleaves this head's own ctx half-0, lagged 4 slots
                    # behind the per-tile E-multiply.
                    r, t = h % 2, h // 2
                    rs = slice(r * 64, (r + 1) * 64)
                    ev_ap = em_d[h].rearrange("(ko ki) q -> ki ko q", ki=P)
                    em_sb = []
                    for j in range(2):
                        ej = ppool.tile([P, 4, S], f16, tag="prev", name=f"em_{h}_{j}")
                        nc.sync.dma_start(ej, ev_ap[:, j * 4 : (j + 1) * 4, :])
                        em_sb.append(ej)

                    probsT = probs_pool.tile(
                        [P, 8, S], f16, tag="probsT", name=f"probsT_{h}"
                    )
                    probsT_live[h] = probsT
                    if hc is not None:
                        ctx_ps_live[hc] = ps_ctx.tile(
                            [65, S], f32, tag="psc", name=f"ps_c_{hc}"
                        )
                    if hc2 is not None:
                        # second ctx stream's PSUM comes from the score pool
                        # (the dedicated ctx slot is held by hc)
                        ctx_ps_live[hc2] = ps_sc.tile(
                            [65, S], f32, tag="pssc", name=f"ps_c_{hc2}"
                        )
                    n_kt = 8
                    for kt in range(n_kt):
                        ks = slice(kt * P, (kt + 1) * P)
                        ps = ps_sc.tile([P, S], f32, tag="pssc", name=f"ps_s_{h}_{kt}")
                        for half in range(2):
                            hs = slice(half * 512, half * 512 + 512)
                            nc.tensor.matmul(
                                ps[:, hs],
                                lhsT=kT[rs, t, ks],
                                rhs=qT[rs, t, hs],
                                start=True,
                                stop=True,
                            )
                        nc.scalar.activation(
                            probsT[:, kt, :], ps[:], Exp, bias=neg_shift
                        )
                        if kt % mult_chunk == mult_chunk - 1:
                            j0 = kt + 1 - mult_chunk
                            cv = probsT[:, j0 : kt + 1, :]
                            ej = em_sb[j0 // 4]
                            nc.vector.tensor_mul(
                                cv, cv, ej[:, j0 % 4 : j0 % 4 + mult_chunk, :]
                            )
                        if hc is not None:
                            emit_ctx_mm(hc, 2 * kt)
                            emit_ctx_mm(hc, 2 * kt + 1)
                        if hc2 is not None and not self_ctx:
                            emit_ctx_mm(hc2, 2 * kt)
                            emit_ctx_mm(hc2, 2 * kt + 1)
                        if self_ctx and kt >= 4:
                            emit_ctx_mm(h, 2 * (kt - 4))
                            emit_ctx_mm(h, 2 * (kt - 4) + 1)
                        # drain PE fill work (projections) for this slot
                        rem = len(fill_q)
                        if rem:
                            rate = -(-rem // (n_kt - kt))
                            rate = min(rate, 11)
                            for _ in range(min(rate, rem)):
                                fill_q.pop(0)()
                    if hc is not None:
                        emit_ctx_tail(hc)
                    if tail2 and hc2 is not None:
                        emit_ctx_tail(hc2)

                # --- schedule ---
                # q0/k0 projected bulk (PE's first work after hidT lands);
                # their weight DMAs are issued before the big hidT transfer
                queue_proj("q", 0, qT)
                hT_v = hT_d.rearrange("(do di) s -> di do s", di=P)
                nc.sync.dma_start(hidT[:, :, 0:512], hT_v[:, :, 0:512])
                nc.sync.dma_start(hidT[:, :, 512:S], hT_v[:, :, 512:S])
                queue_proj("k", 0, kT)
                while fill_q:
                    fill_q.pop(0)()
                queue_v()
                queue_proj("q", 1, qT)
                queue_proj("k", 1, kT)
                for h in range(16):
                    t = h // 2
                    if h >= 2 and h % 2 == 0 and t + 1 <= 7:
                        queue_proj("q", t + 1, qT)
                    if h >= 2 and h % 2 == 1 and t + 1 <= 7:
                        queue_proj("k", t + 1, kT)
                    emit_head(h, h - 2 if h >= 2 else None,
                              mult_chunk=2 if h == 15 else 4)
                ctx_ps_live[14] = ps_ctx.tile([65, S], f32, tag="psc", name="ps_c_14")
                for idx in range(16):
                    emit_ctx_mm(14, idx)
                emit_ctx_tail(14)
                ctx_ps_live[15] = ps_ctx.tile([65, S], f32, tag="psc", name="ps_c_15")
                for idx in range(8):
                    emit_ctx_mm(15, idx)
                # ctx15: copy finished half-0 while half-1 matmuls run
                pc15 = ctx_ps_live[15]
                ctxT_sb = small.tile([65, S], f16, tag="ctxT", name="ctxT_15")
                nc.vector.tensor_copy(ctxT_sb[:, 0:512], pc15[:, 0:512])
                for idx in range(8, 16):
                    emit_ctx_mm(15, idx)
                nc.vector.tensor_copy(ctxT_sb[:, 512:S], pc15[:, 512:S])
                oh = outp.tile([P, 8, 64], f32, tag="outh", name="out_15")
                for qt in range(8):
                    ptt = ps_t.tile([P, 65], f16, tag="pst", name=f"pt_15_{qt}")
                    nc.tensor.matmul(
                        ptt,
                        lhsT=ctxT_sb[:, qt * P : (qt + 1) * P],
                        rhs=ident[0:65, 0:65],
                        is_transpose=True,
                    )
                    rc = small.tile([P, 1], f32, tag="recip", name=f"rc_15_{qt}")
                    nc.vector.reciprocal(rc, ptt[:, 64:65])
                    nc.vector.tensor_scalar_mul(oh[:, qt, :], ptt[:, 0:64], rc)
                nc.sync.dma_start(out_v[:, :, 15 * 64 : 16 * 64], oh)
                probsT_live.pop(15)
                ctx_ps_live.pop(15)

    nc.compile()
    return nc


def _get_compiled(use_bias: bool, reps: int = 1):
    key = (use_bias, reps)
    if key not in _compiled:
        _compiled[key] = _build(use_bias, reps)
    return _compiled[key]


def _prepare_in_maps(
    hidden_states, attn_mask, prev_attn_weights, Wq, bq, Wk, bk, Wv, bv, use_bias
):
    hs = np.asarray(hidden_states, np.float32)
    mask = np.asarray(attn_mask, np.float32)
    prev = np.asarray(prev_attn_weights, np.float32)

    wq16 = (np.asarray(Wq, np.float32) * SCALE).astype(np.float16)
    wk16 = np.asarray(Wk, np.float32).astype(np.float16)
    wv16 = np.asarray(Wv, np.float32).astype(np.float16)

    def _slice_w(w):
        # [D, D] -> [t, ki, ko*128+c] so the per-t weight DMA is contiguous
        return np.ascontiguousarray(
            w.reshape(8, P, 8, P).transpose(2, 1, 0, 3).reshape(8, P, D)
        )

    wqs = _slice_w(wq16)
    wks = _slice_w(wk16)

    # fold mask in, pre-transpose to [b, h, k, q], exponentiate, cast fp16
    if np.any(mask):
        scores_add = prev + mask
    else:
        scores_add = prev
    em = np.exp(
        scores_add.transpose(0, 1, 3, 2).astype(np.float32) - EXP_SHIFT
    ).astype(np.float16)
    hT = np.ascontiguousarray(hs.transpose(0, 2, 1)).astype(np.float16)

    in_maps = []
    for b in range(N_CORES):
        m = {
            "hiddenT": np.ascontiguousarray(hT[b]),
            "wqs": wqs,
            "wks": wks,
            "wv": wv16,
            "em": np.ascontiguousarray(em[b]),
        }
        if use_bias:
            m["bq"] = (np.asarray(bq, np.float32) * SCALE).astype(np.float16)[None, :]
            m["bk"] = np.asarray(bk, np.float32).astype(np.float16)[None, :]
            m["bv"] = np.asarray(bv, np.float32).astype(np.float16)[None, :]
        in_maps.append(m)
    return in_maps


def kernel(hidden_states, attn_mask, prev_attn_weights, Wq, bq, Wk, bk, Wv, bv):
    from concourse.bass_utils import run_bass_kernel_spmd

    use_bias = bool(np.any(bq) or np.any(bk) or np.any(bv))
    nc = _get_compiled(use_bias)
    in_maps = _prepare_in_maps(
        hidden_states, attn_mask, prev_attn_weights, Wq, bq, Wk, bk, Wv, bv, use_bias
    )
    res = run_bass_kernel_spmd(nc, in_maps, core_ids=list(range(N_CORES)))
    return np.stack([res.results[b]["out"] for b in range(N_CORES)]).astype(np.float32)
